# revision 12
# baseline (speedup 1.0000x reference)
"""Trainium2 Bass kernel for a 2-layer LSTM text classifier.

Model (see original nn.Module): embedding lookup -> 2-layer BasicLSTM
(H=100, T=25) -> dense(128) -> dense(2). Batch 512 is data-parallel
across 8 NeuronCores (64 rows/core); all parameters are replicated.
The embedding gather (pure indexing) happens host-side so only the
used rows ship to the devices, pre-transposed to the feature-major
layout the kernel wants. The two dense head layers have no activation
between them, so they are folded host-side into one [100, 2] matmul.

Device kernel design notes:
- Feature-major layout everywhere: [hidden=100 partitions, batch=64
  free]; the recurrence never needs a transpose.
- All four gates go through a single tanh activation per cell:
  sigmoid(x) = (1 + tanh(x/2))/2, with the 1/2 prescale folded into
  the i/f/o weight columns host-side. Keeping the cell state doubled
  (C = 2c) makes the whole cell update exactly 4 scalar_tensor_tensor
  ops + 2 activations:
      p^ = (ti + 1) * tj          # = 2 sigmoid(i) tanh(j)
      q^ = (tf + 1) * C           # = 4 c sigmoid(f+1)
      C' = q^ * 0.5 + p^          # = 2 c'
      tc = tanh(0.5 * C')        # = tanh(c')
      h~ = (to + 1) * tc          # = 2 h   (next-layer weights absorb 0.5)
- Layer-2 runs exactly one pipeline phase behind layer-1, and every
  engine queue is ordered so the matmul group that closes z1(t) (the
  only truly serial dependency, h1(t-1) -> h1(t)) is always at the
  head of the PE queue when its semaphore fires. The layer-2 close,
  the z2 bias/recurrent prep for the NEXT step, and the layer-1 input
  part all fire in the idle gaps.
- Layer-1 bias rides in an extra contraction row of w1x (xt carries a
  ones row at partition 100); layer-2 bias enters via a rank-4 one-hot
  selector matmul that opens each z2 PSUM bank.
- Bulk DMAs ride the gpsimd software-DGE queue, which round-robins
  descriptors across all 16 DMA engines (~100 GB/s observed); the two
  small first-needed tensors take the sync HW queue in parallel. The
  recurrence starts as soon as w1x + xt[t<2] land.
- Matmuls are bf16 (stationary depadded to 100 cols) accumulating in
  fp32 PSUM; cell state stays fp32.
"""

import functools
import os
import sys

import numpy as np

for _p in ("/opt/trn_rl_repo", "/root/.axon_site/_ro/trn_rl_repo"):
    if os.path.isdir(_p) and _p not in sys.path:
        sys.path.insert(0, _p)
        break

import ml_dtypes

from concourse import bass, bass2jax, mybir
from concourse.bass_utils import run_bass_kernel_spmd
from concourse.tile import TileContext

# --- BIR sync-wait rebalancer -------------------------------------------
# The walrus build in this image enforces ONE sync-wait command per ISA
# instruction struct, but Tile's semaphore assignment happily emits 2-4
# waits on matmuls/DVE ops at psum-recycle points. Rewrite the BIR before
# walrus: park one matmul wait on the adjacent Ldweights (same in-order
# queue, executes strictly before the matmul) and split any remaining
# excess onto pure-wait EventSemaphore carriers inserted directly before
# the offending instruction on its own queue. Semantics are unchanged --
# every wait still completes before the instruction it guarded.

_WAIT_PASSTHROUGH = {"EventSemaphore", "UnconditionalBranch", "Call",
                     "RegisterMove", "ISA"}


def _rebalance_bir_waits(bir_bytes):
    import orjson
    bir = orjson.loads(bir_bytes)
    n = 0
    for fn in bir["functions"]:
        for blk in fn["blocks"]:
            out = []
            prev = None
            for inst in blk["instructions"]:
                op = inst.get("opcode")
                si = inst.get("sync_info") or {}
                waits = si.get("on_wait") or []
                if op not in _WAIT_PASSTHROUGH and len(waits) > 1:
                    if (op == "Matmult" and prev is not None
                            and prev.get("opcode") == "Ldweights"
                            and not (prev.get("sync_info") or {}).get("on_wait")):
                        tsi = prev.setdefault("sync_info", {})
                        tsi.setdefault("on_wait", []).append(waits.pop(0))
                    while len(waits) > 1:
                        n += 1
                        out.append({
                            "debug": inst.get("debug", 0),
                            "engine": inst["engine"],
                            "ins": [], "outs": [],
                            "name": f"antwait_{n}",
                            "opcode": "EventSemaphore",
                            "sync_info": {"on_update": [],
                                          "on_wait": [waits.pop(0)]},
                        })
                    si["on_wait"] = waits
                out.append(inst)
                prev = inst
            blk["instructions"] = out
    return orjson.dumps(bir)


_orig_compile_bir_kernel = bass2jax.compile_bir_kernel


def _compile_bir_kernel_rebalanced(bir_json, tmpdir, neff_name="file.neff"):
    return _orig_compile_bir_kernel(_rebalance_bir_waits(bir_json), tmpdir,
                                    neff_name=neff_name)


if bass2jax.compile_bir_kernel is not _compile_bir_kernel_rebalanced:
    bass2jax.compile_bir_kernel = _compile_bir_kernel_rebalanced

H = 100          # hidden size
T = 25           # sequence length
B = 512          # total batch
N_CORES = 8
BC = B // N_CORES  # 64 per-core batch
NCLS = 2         # logits
FORGET_BIAS = 1.0

# xt streams in 4 chunks; chunk k covers time steps [CH[k], CH[k+1])
CH = (0, 2, 6, 14, 25)

BF16 = ml_dtypes.bfloat16
_DT = mybir.dt
TANH = mybir.ActivationFunctionType.Tanh
ADD = mybir.AluOpType.add
MULT = mybir.AluOpType.mult

# gate slot order in PSUM: [i, f, o, j]; source block order in the
# TF BasicLSTMCell kernel is [i, j, f, o]
SLOT_SRC_BLOCK = (0, 2, 3, 1)
SLOT_PRESCALE = (0.5, 0.5, 0.5, 1.0)  # tanh(x/2) trick for i/f/o, plain tanh for j


def _build_nc():
    nc = bass.Bass()
    xt_d = nc.dram_tensor("xt", [H + 1, T * BC], _DT.bfloat16, kind="ExternalInput")
    w1x_d = nc.dram_tensor("w1x", [H + 1, 402], _DT.bfloat16, kind="ExternalInput")
    w1h_d = nc.dram_tensor("w1h", [H, 400], _DT.bfloat16, kind="ExternalInput")
    w2x_d = nc.dram_tensor("w2x", [H, 400], _DT.bfloat16, kind="ExternalInput")
    w2h_d = nc.dram_tensor("w2h", [H, 400], _DT.bfloat16, kind="ExternalInput")
    b2_d = nc.dram_tensor("b2p", [4, 358], _DT.bfloat16, kind="ExternalInput")
    out_d = nc.dram_tensor("out", [NCLS, BC], _DT.float32, kind="ExternalOutput")

    with TileContext(nc) as tc:
        with tc.tile_pool(name="const", bufs=1) as cpool, \
             tc.tile_pool(name="work", bufs=3) as wpool, \
             tc.tile_pool(name="psum", bufs=3, space="PSUM") as ppool, \
             tc.tile_pool(name="psfc", bufs=1, space="PSUM") as fpool:

            # warm the tanh table on ACT while DMAs run
            scratch = cpool.tile([1, 1], _DT.float32, tag="scratch")
            nc.vector.memset(scratch[:, :], 0.0)
            nc.scalar.activation(scratch[:, :], scratch[:, :], TANH)

            b2s = cpool.tile([4, 358], _DT.bfloat16, tag="b2s")
            w1x = cpool.tile([H + 1, 402], _DT.bfloat16, tag="w1x")
            w1h = cpool.tile([H, 400], _DT.bfloat16, tag="w1h")
            w2x = cpool.tile([H, 400], _DT.bfloat16, tag="w2x")
            w2h = cpool.tile([H, 400], _DT.bfloat16, tag="w2h")
            xts = [cpool.tile([H + 1, (CH[k + 1] - CH[k]) * BC], _DT.bfloat16,
                              tag=f"xt{k}", name=f"xt{k}") for k in range(4)]
            ones = cpool.tile([1, BC], _DT.bfloat16, tag="ones")
            b2t = b2s[0:4, 0:H]
            b2sel = b2s[0:4, 100:356]
            bful = b2s[0:1, 356:358]
            wcomb = w1x[0:H, 400:402]

            # DMA priority order: the two small first-needed tensors on
            # the sync HW-DGE queue; the layer-1 weights on the scalar
            # HW-DGE queue (observed to spread packets across ~10 DMA
            # engines); the rest on the gpsimd SW-DGE queue, which
            # round-robins across all 16 engines.  The recurrence can
            # start once b2s + xt0 + w1x + w1h have landed (~190 KB).
            nc.sync.dma_start(out=w1x[:, :], in_=w1x_d[:, :])
            nc.sync.dma_start(out=xts[0][:, :], in_=xt_d[:, 0:CH[1] * BC])
            nc.sync.dma_start(out=b2s[:, :], in_=b2_d[:, :])
            nc.scalar.dma_start(out=w1h[:, :], in_=w1h_d[:, :])
            nc.gpsimd.dma_start(out=w2x[:, :], in_=w2x_d[:, :])
            nc.gpsimd.dma_start(out=w2h[:, :], in_=w2h_d[:, :])
            nc.gpsimd.dma_start(out=xts[1][:, :],
                                in_=xt_d[:, CH[1] * BC:CH[2] * BC])
            nc.sync.dma_start(out=xts[2][:, :],
                              in_=xt_d[:, CH[2] * BC:CH[3] * BC])
            nc.sync.dma_start(out=xts[3][:, :],
                              in_=xt_d[:, CH[3] * BC:CH[4] * BC])
            nc.vector.memset(ones[:, :], 1.0)

            # ping/pong recurrent state.  Step 0 of each layer skips the
            # recurrent matmuls and the q^ term entirely (zero state), so
            # nothing needs zero-init.
            h1 = [cpool.tile([H, BC], _DT.bfloat16, tag=f"h1_{i}", name=f"h1_{i}") for i in range(2)]
            h2 = [cpool.tile([H, BC], _DT.bfloat16, tag=f"h2_{i}", name=f"h2_{i}") for i in range(2)]
            c1 = [cpool.tile([H, BC], _DT.float32, tag=f"c1_{i}", name=f"c1_{i}") for i in range(2)]
            c2 = [cpool.tile([H, BC], _DT.float32, tag=f"c2_{i}", name=f"c2_{i}") for i in range(2)]

            def xsrc(t):
                for k in range(4):
                    if t < CH[k + 1]:
                        return xts[k], (t - CH[k]) * BC
                raise AssertionError

            def x_part(z1, t, close):
                # layer-1 input contribution + bias (via xt's ones row);
                # opens the z1(t) PSUM bank
                xt, col = xsrc(t)
                for g in range(4):
                    nc.tensor.matmul(
                        z1[0:H, g * BC:(g + 1) * BC],
                        lhsT=w1x[0:H + 1, g * H:(g + 1) * H],
                        rhs=xt[0:H + 1, col:col + BC],
                        start=(g == 0), stop=(close and g == 3))

            def close1(z1, h_rd):
                for g in range(4):
                    nc.tensor.matmul(
                        z1[0:H, g * BC:(g + 1) * BC],
                        lhsT=w1h[0:H, g * H:(g + 1) * H],
                        rhs=h_rd[0:H, :],
                        start=False, stop=(g == 3))

            def close2(z2, h_rd):
                for g in range(4):
                    nc.tensor.matmul(
                        z2[0:H, g * BC:(g + 1) * BC],
                        lhsT=w2x[0:H, g * H:(g + 1) * H],
                        rhs=h_rd[0:H, :],
                        start=False, stop=(g == 3))

            def open2(z2, h2_rd):
                # z2 bank opener reads only constants: rank-4 matmul
                # broadcasts each gate's bias row into its slot via a
                # one-hot selector; then the recurrent h2 contribution
                nc.tensor.matmul(z2[0:H, 0:256], lhsT=b2t, rhs=b2sel,
                                 start=True, stop=False)
                if h2_rd is not None:
                    for g in range(4):
                        nc.tensor.matmul(
                            z2[0:H, g * BC:(g + 1) * BC],
                            lhsT=w2h[0:H, g * H:(g + 1) * H],
                            rhs=h2_rd[0:H, :],
                            start=False, stop=False)

            def cell(z, c_rd, c_wr, h_wr, tag, first):
                # gates -> new cell state / hidden, all in [H, BC] layout
                tg = wpool.tile([128, 256], _DT.bfloat16, tag=f"t{tag}", name=f"t{tag}")
                nc.scalar.activation(tg[0:H, :], z[0:H, 0:256], TANH)
                ti, tf = tg[0:H, 0:64], tg[0:H, 64:128]
                to, tj = tg[0:H, 128:192], tg[0:H, 192:256]
                if first:
                    # C == 0: C' = p^
                    nc.vector.scalar_tensor_tensor(c_wr[:, :], ti, 1.0, tj, op0=ADD, op1=MULT)
                else:
                    qh = wpool.tile([H, BC], _DT.float32, tag=f"q{tag}", name=f"q{tag}")
                    nc.vector.scalar_tensor_tensor(qh[:, :], tf, 1.0, c_rd[:, :], op0=ADD, op1=MULT)
                    ph = wpool.tile([H, BC], _DT.bfloat16, tag=f"p{tag}", name=f"p{tag}")
                    nc.vector.scalar_tensor_tensor(ph[:, :], ti, 1.0, tj, op0=ADD, op1=MULT)
                    nc.vector.scalar_tensor_tensor(c_wr[:, :], qh[:, :], 0.5, ph[:, :], op0=MULT, op1=ADD)
                tcg = wpool.tile([H, BC], _DT.bfloat16, tag=f"tc{tag}", name=f"tc{tag}")
                nc.scalar.activation(tcg[:, :], c_wr[:, :], TANH, scale=0.5)
                nc.vector.scalar_tensor_tensor(h_wr[0:H, :], to, 1.0, tcg[:, :], op0=ADD, op1=MULT)

            # ---- software pipeline (v2 schedule: both layers advance in
            # the same iteration; layer-2's z2 prep for the next step is
            # emitted last so it fires in the PE queue's idle tail) ----
            z1 = ppool.tile([128, 256], _DT.float32, tag="z1")
            x_part(z1, 0, close=True)
            z2 = ppool.tile([128, 256], _DT.float32, tag="z2")
            open2(z2, None)                      # z2(0): bias only, h2(-1)=0

            for t in range(T):
                rd, wr = (t + 1) % 2, t % 2
                if t > 0:
                    close1(z1, h1[rd])            # PE: critical path
                cell(z1, c1[rd], c1[wr], h1[wr], "1", first=(t == 0))
                if t + 1 < T:
                    z1 = ppool.tile([128, 256], _DT.float32, tag="z1")
                    x_part(z1, t + 1, close=False)
                close2(z2, h1[wr])                # PE: closes z2(t)
                cell(z2, c2[rd], c2[wr], h2[wr], "2", first=(t == 0))
                if t + 1 < T:
                    z2 = ppool.tile([128, 256], _DT.float32, tag="z2")
                    open2(z2, h2[wr])             # PE: preps z2(t+1)

            lw = (T - 1) % 2
            predp = fpool.tile([NCLS, BC], _DT.float32, tag="pred", name="predp")
            nc.tensor.matmul(predp[0:NCLS, :], lhsT=bful, rhs=ones[0:1, :],
                             start=True, stop=False)
            nc.tensor.matmul(predp[0:NCLS, :], lhsT=wcomb,
                             rhs=h2[lw][0:H, :], start=False, stop=True)
            outs = wpool.tile([NCLS, BC], _DT.float32, tag="outs")
            nc.vector.tensor_copy(outs[:, :], predp[0:NCLS, :])
            nc.sync.dma_start(out=out_d[:, :], in_=outs[:, :])

    return nc


@functools.lru_cache(maxsize=1)
def _get_nc():
    return _build_nc()


def _scaled_gate_blocks(kmat, rows, extra_scale):
    """[rows x 400] bf16 tile: gate blocks reordered to [i,f,o,j],
    prescaled for the tanh-only gate trick."""
    out = np.zeros((rows.stop - rows.start, 400), np.float32)
    for slot in range(4):
        b = SLOT_SRC_BLOCK[slot]
        out[:, slot * H:(slot + 1) * H] = (
            kmat[rows, b * H:(b + 1) * H] * (SLOT_PRESCALE[slot] * extra_scale))
    return out


def _prep_weights(k1, b1, k2, b2, w_fc1, b_fc1, w_fc2, b_fc2):
    w1x = np.zeros((H + 1, 402), np.float32)
    w1x[0:H, 0:400] = _scaled_gate_blocks(k1, slice(0, H), 1.0)
    b2p = np.zeros((4, 358), np.float32)
    for slot in range(4):
        b = SLOT_SRC_BLOCK[slot]
        fb = FORGET_BIAS if slot == 1 else 0.0
        w1x[H, slot * H:(slot + 1) * H] = (b1[b * H:(b + 1) * H] + fb) * SLOT_PRESCALE[slot]
        b2p[slot, 0:H] = (b2[b * H:(b + 1) * H] + fb) * SLOT_PRESCALE[slot]
        b2p[slot, 100 + slot * 64:100 + (slot + 1) * 64] = 1.0
    # folded dense head: pred = h2~ @ (0.5 w_fc1 w_fc2) + (b_fc1 w_fc2 + b_fc2)
    wcomb = (0.5 * w_fc1.astype(np.float64)) @ w_fc2.astype(np.float64)
    bful = b_fc1.astype(np.float64) @ w_fc2.astype(np.float64) + b_fc2
    w1x[0:H, 400:402] = wcomb.astype(np.float32)
    b2p[0, 356:358] = bful.astype(np.float32)
    return {
        "w1x": w1x.astype(BF16),
        "w1h": _scaled_gate_blocks(k1, slice(H, 2 * H), 0.5).astype(BF16),
        "w2x": _scaled_gate_blocks(k2, slice(0, H), 0.5).astype(BF16),
        "w2h": _scaled_gate_blocks(k2, slice(H, 2 * H), 0.5).astype(BF16),
        "b2p": b2p.astype(BF16),
    }


def _run(inputs, trace=False):
    nc = _get_nc()
    feats = np.asarray(inputs["features"])
    x = np.asarray(inputs["embedding"])[feats]          # [B, T, H] host gather
    shared = _prep_weights(
        np.asarray(inputs["k1"]), np.asarray(inputs["b1"]),
        np.asarray(inputs["k2"]), np.asarray(inputs["b2"]),
        np.asarray(inputs["w_fc1"]), np.asarray(inputs["b_fc1"]),
        np.asarray(inputs["w_fc2"]), np.asarray(inputs["b_fc2"]))
    in_maps = []
    for c in range(N_CORES):
        xt = np.ones((H + 1, T * BC), np.float32)
        # [BC, T, H] -> [H, T, BC] feature-major with a trailing ones row
        xt[0:H] = x[c * BC:(c + 1) * BC].transpose(2, 1, 0).reshape(H, T * BC)
        in_maps.append({**shared, "xt": xt.astype(BF16)})
    res = run_bass_kernel_spmd(nc, in_maps, core_ids=list(range(N_CORES)),
                               trace=trace)
    out = np.empty((B, NCLS), np.float32)
    for c in range(N_CORES):
        out[c * BC:(c + 1) * BC] = res.results[c]["out"].T
    return out, res


def kernel(**inputs):
    out, _ = _run(inputs, trace=False)
    return out
